# revision 38
# baseline (speedup 1.0000x reference)
"""Multihead self-attention (T=1024, B=4, E=1024, H=16) on 8 TRN2 NeuronCores.

Sharding: head-parallel. Core i owns heads {2i, 2i+1} == E-rows [128i, 128i+128)
of Wq/Wk/Wv, and all 4 batches. No cross-core communication.

v5 design (v2 + startup/tail restructure; ~93.1-93.8us HW, vs 95.2us for
the v2 baseline re-measured on the same machine):
  - ALL input DMAs are issued up-front on the sync HWDGE ring in strict
    consumption order (wq, qu0a, bqkv, qu0b, wk, wv, qu1..qu7); band/pad
    ride gpsimd SWDGE. Keeping first-needed data off the scalar HWDGE
    queue matters: that queue starts ~4us later than sync.
  - qu chunk 0 is DMA'd in two halves so the first projection matmuls can
    start after 512 KB instead of 1 MB.
  - PE warmup transposes read a gpsimd-memset tile (no make_identity
    dep), so they start as soon as the engines boot (~6.9us) and keep the
    PE continuously busy through the DMA fill; the HAM grants the full
    2.4 GHz clock at ~activity_start + 9us, so any activity gap before
    the grant delays it (measured: 16 warmups ends right as qu0 lands;
    both fewer and more warmups regressed).
  - only set1/q-j0/k-j0 of batch 0 run before attention; the rest of
    batch-0's projection drains into chunk (0,0)'s score slots like every
    other batch's does, so scores start ~4us earlier.
  - last chunk's two per-head output DMAs go out on different engines
    (sync + scalar) so their issue costs overlap.

Measured dead ends (do not retry blindly): fp8 DoubleRow projections
(numerically fine with a 3-term main+residual scheme, rel-err 5.3e-3,
but DR matmuls run at ~1 cycle per MOVING row on HW = no speedup + more
LDWEIGHTS); v-transpose via dma_start_transpose (~1.2us per issue on the
sync queue, va tiles arrive ~1us late at every chunk); mid-drain partial
po copies (bounding-box PSUM hazards serialize the final AVs).

Per-core per-chunk dataflow (unchanged from v2):
  qT/kT/vT [128, t] = W_slice @ query.T  (PE, bf16, K=E in 8 chunks, fp32
  PSUM, bias added on the psum->SBUF copy by the ACT)
  va [t(s), head, 65] = PE-transpose of vT, with a ones column at index 64.
  per (b, t-chunk c of 512):
    scoresT [s=128, t] per s-tile p (PE, skipping above-diagonal and
    fully-padded tiles); exp with per-partition padding bias (ACT);
    0/1 band mask on the diagonal 128 cols (DVE);
    po [65, 2, t] += va.T @ probs (PE; row 64 = denominator);
    po -> SBUF bf16 copy (DVE) -> DMA out.
  Output normalize (num/den) + transpose happens on the host in numpy.
"""

import numpy as np

T, B, E, H = 1024, 4, 1024, 16
D = 64  # head dim
NCORES = 8
HPC = H // NCORES  # heads per core = 2
DS = HPC * D  # per-core E-slice = 128
NEG = -1.0e30
SCALE = D**-0.5
N_WARM = 26  # PE warmup transposes (clock ramp + DMA-fill cover)

_COMPILED = {}


def _build_program(S):
    import concourse.bacc as bacc
    import concourse.mybir as mybir
    import concourse.tile as tile
    from concourse.masks import make_identity

    f32 = mybir.dt.float32
    bf16 = mybir.dt.bfloat16
    AF = mybir.ActivationFunctionType
    ALU = mybir.AluOpType

    nc = bacc.Bacc("TRN2", target_bir_lowering=False, debug=False,
                   num_devices=NCORES)

    # qt pre-tiled on the host: [chunk j, p, c, t] so each chunk DMA is one
    # contiguous 8KB run per partition (128 descriptors instead of 1024)
    qt = nc.dram_tensor("qt", [2 * B, 128, 8, 512], bf16,
                        kind="ExternalInput").ap()
    wq = nc.dram_tensor("wq", [128, 8, DS], bf16, kind="ExternalInput").ap()
    wk = nc.dram_tensor("wk", [128, 8, DS], bf16, kind="ExternalInput").ap()
    wv = nc.dram_tensor("wv", [128, 8, DS], bf16, kind="ExternalInput").ap()
    bqkv = nc.dram_tensor("bqkv", [DS, 3], f32, kind="ExternalInput").ap()
    band = nc.dram_tensor("band", [128, 128], bf16, kind="ExternalInput").ap()
    # additive exp bias per (s-partition, batch, key-tile): 0 for valid
    # keys, -1e30 for padded ones (exp -> 0)
    pad = nc.dram_tensor("pad", [128, B * 8], f32, kind="ExternalInput").ap()
    # [b, chunk, d-or-den, head, t] with the denominator in row 64
    out = nc.dram_tensor("out", [B, 2, 65, HPC, 512], bf16,
                         kind="ExternalOutput").ap()

    with tile.TileContext(nc) as tc:
        with (
            tc.tile_pool(name="consts", bufs=1) as consts,
            tc.tile_pool(name="qup", bufs=8) as qup,
            tc.tile_pool(name="qkp", bufs=2) as qkp,
            tc.tile_pool(name="vtp", bufs=2) as vtp,
            tc.tile_pool(name="vap", bufs=2) as vap,
            tc.tile_pool(name="prp", bufs=18) as prp,
            tc.tile_pool(name="posp", bufs=3) as posp,
            # psA double-buffered so the next projection unit's matmuls
            # don't wait the previous unit's ACT bias-copy; the transpose
            # tiles ride inside psS's "sc" tag footprint (tags are sized by
            # the max tile, so a [128,128]bf16 costs no extra bank there)
            tc.tile_pool(name="psA", bufs=2, space="PSUM") as psA,
            tc.tile_pool(name="psS", bufs=2, space="PSUM") as psS,
            tc.tile_pool(name="psO", bufs=1, space="PSUM") as psO,
        ):
            # ---- warmup stationary: memset only (no make_identity dep) so
            # the PE can start ramping the HAM clock immediately; gpsimd
            # finishes its preamble first so it does the memset ----
            warm = consts.tile([128, 128], bf16, name="warm")
            nc.gpsimd.memset(warm[:], 1.0)

            # ---- all input DMAs issued up-front on three queues ----
            w_sb = {}
            b_sb = {}
            for nm, wdr in (("q", wq), ("k", wk), ("v", wv)):
                w_sb[nm] = consts.tile([128, 8, DS], bf16, name=f"w{nm}s")
            bqkv_sb = consts.tile([DS, 3], f32, name="bqkvs")
            for i, nm in enumerate(("q", "k", "v")):
                b_sb[nm] = bqkv_sb[:, i:i + 1]
            band_sb = consts.tile([128, 128], bf16, name="bands")
            pad_sb = consts.tile([128, B * 8], f32, name="pads")

            qu_t = [qup.tile([128, 8, 512], bf16, tag="qu", name=f"qu{j}")
                    for j in range(2 * B)]
            # single sync ring, strict consumption order (the scalar HWDGE
            # queue starts ~4us later than sync, so putting anything
            # first-needed there stalls the first projection)
            nc.sync.dma_start(w_sb["q"][:], wq)
            nc.sync.dma_start(qu_t[0][:, 0:4, :], qt[0][:, 0:4, :])
            nc.sync.dma_start(bqkv_sb[:], bqkv)
            nc.sync.dma_start(qu_t[0][:, 4:8, :], qt[0][:, 4:8, :])
            nc.sync.dma_start(w_sb["k"][:], wk)
            nc.sync.dma_start(w_sb["v"][:], wv)
            for j in range(1, 2 * B):
                nc.sync.dma_start(qu_t[j][:], qt[j])
            # gpsimd (SWDGE): small mask constants
            nc.gpsimd.dma_start(band_sb[:], band)
            nc.gpsimd.dma_start(pad_sb[:], pad)

            # PE warmup: dummy transposes while the input DMAs are in
            # flight. Trips the HAM activity monitor so the real projection
            # starts at 2.4 GHz.
            for wi in range(N_WARM):
                tpw = psS.tile([128, 128], bf16, tag="sc", name=f"warm{wi}")
                nc.tensor.transpose(tpw[:], warm[:], warm[:])

            ident = consts.tile([128, 128], bf16, name="ident")
            make_identity(nc, ident[:])

            # ---- per-batch persistent tiles ----
            qk_t = {}   # (nm, b) -> [128, 1024] bf16
            vt_t = {}   # b -> [128, 1024] bf16 (vT, bias applied)
            va_t = {}   # b -> [128, 8, HPC, 65] bf16

            # ---- projection work units for batch b ----
            # Unit order: [set1, q-j0, k-j0, v-j0, vtr 0..3, q-j1, k-j1,
            # v-j1, vtr 4..] — all j0 units precede j1 units so a suffix of
            # the list can be deferred into a batch's own attention chunks
            # (used for batch 0 and the last batch).
            def proj_unit(nm, b, j, N, dst, nsub=1):
                def emit():
                    ps = psA.tile([128, 512], f32, tag="proj",
                                  name=f"ps{nm}{b}{j}")
                    qu = qu_t[2 * b + j]
                    for e in range(8):
                        nc.tensor.matmul(
                            ps[:, 0:N],
                            w_sb[nm][:, e, :],
                            qu[:, e, 0:N],
                            start=(e == 0),
                            stop=(e == 7),
                        )
                    nc.scalar.activation(
                        dst[:, 512 * j:512 * j + N], ps[:, 0:N],
                        AF.Identity, bias=b_sb[nm], scale=1.0,
                    )

                return emit

            def vtr_unit(b, i):
                def emit():
                    tp = psS.tile([128, 128], bf16, tag="sc",
                                  name=f"tp{b}_{i}")
                    nc.tensor.transpose(
                        tp[:], vt_t[b][:, 128 * i:128 * (i + 1)], ident[:],
                    )
                    nc.vector.tensor_copy(
                        va_t[b][:, i, :, 0:64],
                        tp[:].rearrange("p (two sub) -> p two sub", two=2),
                    )

                return emit

            def proj_units(b):
                dsts = {}
                for nm in ("q", "k"):
                    dsts[nm] = qkp.tile([128, T], bf16, tag=nm,
                                        name=f"{nm}{b}")
                    qk_t[(nm, b)] = dsts[nm]
                dsts["v"] = vtp.tile([128, T], bf16, tag="vt", name=f"vt{b}")
                vt_t[b] = dsts["v"]
                va = vap.tile([128, 8, HPC, 65], bf16, tag="va",
                              name=f"va{b}")
                va_t[b] = va
                units = [lambda: nc.vector.memset(va[:, :, :, 64:65], 1.0)]
                for j in range(2):
                    for nm in ("q", "k", "v"):
                        ncols = T if nm == "q" else min(T, 128 * S[b])
                        N = min(512, ncols - 512 * j)
                        if N > 0:
                            units.append(proj_unit(nm, b, j, N, dsts[nm]))
                    for i in range(4 * j, min(4 * (j + 1), S[b])):
                        units.append(vtr_unit(b, i))
                return units

            # ---- attention emission ----
            def emit_scores(b, c, p):
                w0 = max(0, 128 * (p - 4 * c))
                ss = psS.tile([128, HPC, 512], f32, tag="sc",
                              name=f"sc{b}_{c}_{p}")
                kt = qk_t[("k", b)]
                qt_b = qk_t[("q", b)]
                for hl in range(HPC):
                    nc.tensor.matmul(
                        ss[:, hl, w0:512],
                        kt[64 * hl:64 * hl + 64, 128 * p:128 * p + 128],
                        qt_b[64 * hl:64 * hl + 64,
                             512 * c + w0:512 * (c + 1)],
                        start=True,
                        stop=True,
                    )
                pr = prp.tile([128, HPC, 512], bf16, tag="pr",
                              name=f"pr{b}_{c}_{p}")
                nc.scalar.activation(
                    pr[:, :, w0:512],
                    ss[:, :, w0:512],
                    AF.Exp,
                    bias=pad_sb[:, b * 8 + p:b * 8 + p + 1],
                    scale=1.0,
                )
                dlt = p - 4 * c
                if dlt >= 0:
                    nc.vector.tensor_tensor(
                        pr[:, :, w0:w0 + 128],
                        pr[:, :, w0:w0 + 128],
                        band_sb[:, None, :].to_broadcast((128, HPC, 128)),
                        ALU.mult,
                    )
                return pr

            def emit_av(b, c, p, ntile, pr, po):
                w0 = max(0, 128 * (p - 4 * c))
                for hl in range(HPC):
                    nc.tensor.matmul(
                        po[:, hl, w0:512],
                        va_t[b][:, p, hl, :],
                        pr[:, hl, w0:512],
                        start=(p == 0),
                        stop=(p == ntile - 1),
                    )

            def emit_epilogue(b, c, po, last=False):
                pos = posp.tile([65, HPC, 512], bf16, tag="pos",
                                name=f"pos{b}_{c}")
                if last:
                    nc.scalar.copy(pos[:], po[:])
                else:
                    nc.vector.tensor_copy(pos[:], po[:])
                # last chunk: HWDGE (sync) has ~1.4us lower first-byte
                # latency than the gpsimd SWDGE path; mid-kernel chunks stay
                # on the idle gpsimd queue
                eng = nc.sync if last else nc.gpsimd
                eng.dma_start(out[b, c], pos[:])

            # ---------------- schedule ----------------
            # batch 0: only set1/q-j0/k-j0 run before attention (chunk
            # (0,0)'s scores need just those); the rest defers into the
            # chunk slots like every other batch's projection does.
            u0 = proj_units(0)
            for fn in u0[:3]:
                fn()
            punits = u0[3:]
            # units of u0[3:] that must complete before (0,1)'s scores:
            # v-j0, vtr0..min(4,S0)-1, q-j1, k-j1
            pops_before_01 = 3 + min(4, S[0])
            npops = [0]

            def pop_unit():
                punits.pop(0)()
                npops[0] += 1

            chunks = [(b, c) for b in range(B) for c in range(2)]
            prev = None           # (b, c, po)
            prev_pending = []     # [(b, c, p, ntile, pr, po)]
            punits_hold = []      # last batch's j1 units, deferred into its
            punits_hold_c1 = []
            for (b, c) in chunks:  # own attention chunks (pipeline drain)
                if c == 0:
                    if b > 0:
                        while punits:  # leftovers must land before b's
                            pop_unit()  # scores
                    if b + 1 < B:
                        nxt = proj_units(b + 1)
                        if b + 1 == B - 1:
                            # defer everything not needed by the last
                            # batch's own scores: v-j0+vtr0..3+q-j1+k-j1
                            # drain during its c=0 chunk, v-j1+vtr4.. during
                            # c=1 (AV consumption lags a chunk behind)
                            idx1 = 4 + min(4, S[b + 1])  # end of j0 block
                            punits_hold = nxt[3:idx1 + 2]
                            punits_hold_c1 = nxt[idx1 + 2:]
                            nxt = nxt[:3]
                        punits += nxt
                    else:
                        punits += punits_hold
                elif b == 0:
                    # (0,1): batch-0's q-j1/k-j1 must land before scores
                    while npops[0] < pops_before_01:
                        pop_unit()
                elif b + 1 >= B:
                    # last batch, c=1: its scores need q-j1/k-j1 NOW
                    while punits:
                        pop_unit()
                    punits += punits_hold_c1
                ntile = min(4 * (c + 1), S[b])
                last_chunk = (b, c) == chunks[-1]
                po = psO.tile([65, HPC, 512], f32, tag="po",
                              name=f"po{b}_{c}")
                prs = []
                for p in range(ntile):
                    pr = emit_scores(b, c, p)
                    if punits:
                        pop_unit()
                    if prev_pending:
                        emit_av(*prev_pending.pop(0))
                    if last_chunk:
                        # drain the previous chunk at 2 AVs/slot and lag our
                        # own AVs only 3 tiles behind: shortens the pipeline
                        # drain after the final scores tile
                        if prev_pending:
                            emit_av(*prev_pending.pop(0))
                        if not prev_pending and prev is not None:
                            emit_epilogue(*prev)
                            prev = None
                    elif p == 2 and prev is not None:
                        while prev_pending:
                            emit_av(*prev_pending.pop(0))
                        emit_epilogue(*prev)
                        prev = None
                    prs.append((b, c, p, ntile, pr, po))
                    if last_chunk and p >= 3:
                        emit_av(*prs[p - 3])
                if last_chunk:
                    while prev_pending:  # safety for tiny chunk counts
                        emit_av(*prev_pending.pop(0))
                    if prev is not None:
                        emit_epilogue(*prev)
                    rem = prs[max(0, ntile - 3):]
                    for args in rem[:-1]:
                        emit_av(*args)
                    # final tile: per-head AV -> copy -> DMA; the two DMA
                    # issues go on different engines so they overlap
                    (_, _, p_l, nt_l, pr_l, po_l) = rem[-1]
                    w0l = max(0, 128 * (p_l - 4 * c))
                    pos = posp.tile([65, HPC, 512], bf16, tag="pos",
                                    name=f"pos{b}_{c}")
                    for hl in range(HPC):
                        nc.tensor.matmul(
                            po_l[:, hl, w0l:512],
                            va_t[b][:, p_l, hl, :],
                            pr_l[:, hl, w0l:512],
                            start=(p_l == 0),
                            stop=True,
                        )
                        if hl == HPC - 1:
                            # last head sits on the exec critical path:
                            # split the copy across ACT+DVE so the DMA can
                            # start ~260ns sooner
                            nc.scalar.copy(pos[:, hl, 0:256],
                                           po_l[:, hl, 0:256])
                            nc.vector.tensor_copy(pos[:, hl, 256:512],
                                                  po_l[:, hl, 256:512])
                        else:
                            nc.scalar.copy(pos[:, hl, :], po_l[:, hl, :])
                        # both DMAs on the warm sync queue (first use of the
                        # scalar HWDGE queue costs ~1.4us and extends the
                        # teardown)
                        nc.sync.dma_start(out[b, c, :, hl, :], pos[:, hl, :])
                    prev = None
                    prev_pending = []
                else:
                    while prev_pending:
                        emit_av(*prev_pending.pop(0))
                        if punits:
                            pop_unit()
                    if prev is not None:
                        emit_epilogue(*prev)
                    prev = (b, c, po)
                    prev_pending = prs

    nc.compile()
    return nc


def _get_program(S):
    S = tuple(S)
    if S not in _COMPILED:
        _COMPILED[S] = _build_program(S)
    return _COMPILED[S]


def _prepare_in_maps(query, key_padding_mask, attn_mask, Wq, bq, Wk, bk, Wv,
                     bv):
    import ml_dtypes

    bf16 = ml_dtypes.bfloat16

    query = np.asarray(query, dtype=np.float32)
    attn_mask = np.asarray(attn_mask, dtype=np.float32)
    kpm = np.asarray(key_padding_mask)
    Wq, Wk, Wv = (np.asarray(w, dtype=np.float32) for w in (Wq, Wk, Wv))
    bq, bk, bv = (np.asarray(x, dtype=np.float32) for x in (bq, bk, bv))

    Wq = Wq * SCALE
    bq = bq * SCALE
    # [T, B, E] -> [E, B, T] -> [(8c, 128p), B, (2j2, 512t)] -> chunk-major
    # [B*2, 128, 8, 512] so each per-core chunk DMA is fully contiguous
    qt = query.transpose(2, 1, 0).reshape(8, 128, B, 2, 512)
    qt = np.ascontiguousarray(qt.transpose(2, 3, 1, 0, 4))
    qt = qt.reshape(2 * B, 128, 8, 512).astype(bf16)

    # number of 128-wide key tiles per batch that contain >= 1 valid key
    valid = ~kpm  # [B, T], True = usable key
    vt = valid.reshape(B, 8, 128).any(axis=2)  # [B, 8]
    S = []
    for b in range(B):
        nz = np.nonzero(vt[b])[0]
        S.append(int(nz[-1]) + 1 if len(nz) else 1)
    S = tuple(S)

    # 0/1 multiplicative causal mask for the diagonal 128x128 band:
    # band[ps, ft] = 1 iff attn_mask[ft, ps] is finite (ft, ps < 128)
    band = np.isfinite(attn_mask[:128, :128].T).astype(np.float32)
    band = np.ascontiguousarray(band).astype(bf16)

    padf = np.where(kpm, NEG, 0.0).astype(np.float32)  # [B, T]
    padh = np.ascontiguousarray(
        padf.reshape(B, 8, 128).transpose(2, 0, 1).reshape(128, B * 8)
    )

    def wtile(w):  # [DS, E] -> [E, DS] -> [128p, 8c, DS] pre-tiled
        wt = w.T.reshape(8, 128, DS).transpose(1, 0, 2)
        return np.ascontiguousarray(wt).astype(bf16)

    in_maps = []
    for i in range(NCORES):
        rows = slice(i * DS, (i + 1) * DS)
        in_maps.append(
            {
                "qt": qt,
                "wq": wtile(Wq[rows]),
                "wk": wtile(Wk[rows]),
                "wv": wtile(Wv[rows]),
                "bqkv": np.ascontiguousarray(
                    np.stack([bq[rows], bk[rows], bv[rows]], axis=1)),
                "band": band,
                "pad": padh,
            }
        )
    return in_maps, S


def kernel(query, key, key_padding_mask, attn_mask, Wq, bq, Wk, bk, Wv, bv,
           num_heads):
    from concourse.bass_utils import run_bass_kernel_spmd

    assert int(num_heads) == H
    in_maps, S = _prepare_in_maps(query, key_padding_mask, attn_mask, Wq, bq,
                                  Wk, bk, Wv, bv)
    nc = _get_program(S)
    res = run_bass_kernel_spmd(nc, in_maps, core_ids=list(range(NCORES)))
    # res: per-core out [B, 2, 65, HPC, 512] bf16
    full = np.empty((T, B, E), dtype=np.float32)
    for i in range(NCORES):
        arr = np.asarray(res.results[i]["out"], dtype=np.float32)
        num = arr[:, :, 0:64]        # [B, C, D, HL, TL]
        den = arr[:, :, 64:65]       # [B, C, 1, HL, TL]
        r = num / den                # [B, C, D, HL, TL]
        # -> [C, TL, B, HL, D] -> [T, B, 128]
        r = r.transpose(1, 4, 0, 3, 2).reshape(T, B, DS)
        full[:, :, i * DS:(i + 1) * DS] = r
    return full


# revision 41
# speedup vs baseline: 1.0348x; 1.0348x over previous
"""Multihead self-attention (T=1024, B=4, E=1024, H=16) on 8 TRN2 NeuronCores.

Sharding: head-parallel. Core i owns heads {2i, 2i+1} == E-rows [128i, 128i+128)
of Wq/Wk/Wv, and all 4 batches. No cross-core communication.

v6 design (v2 + startup/tail restructure + PSUM re-banking; ~90.5us HW,
vs 95.2us for the v2 baseline re-measured on the same machine):
  - psA (projection PSUM) is double-buffered so a unit's matmuls never
    wait the previous unit's ACT bias-copy (removed 3 stalls of
    ~400-800ns each). The bank for this comes from folding the
    warmup/v-transpose tiles into psS's "sc" tag: pool tags are sized by
    their LARGEST tile, so a [128,128]bf16 transpose target inside the
    [128,2,512]f32 scores tag costs zero extra banks.
  - ALL input DMAs are issued up-front on the sync HWDGE ring in strict
    consumption order (wq, qu0a, bqkv, qu0b, wk, wv, qu1..qu7); band/pad
    ride gpsimd SWDGE. Keeping first-needed data off the scalar HWDGE
    queue matters: that queue starts ~4us later than sync.
  - qu chunk 0 is DMA'd in two halves so the first projection matmuls can
    start after 512 KB instead of 1 MB.
  - PE warmup transposes read a gpsimd-memset tile (no make_identity
    dep), so they start as soon as the engines boot (~6.9us) and keep the
    PE continuously busy through the DMA fill; the HAM grants the full
    2.4 GHz clock at ~activity_start + 9us, so any activity gap before
    the grant delays it (measured: 16 warmups ends right as qu0 lands;
    both fewer and more warmups regressed).
  - only set1/q-j0/k-j0 of batch 0 run before attention; the rest of
    batch-0's projection drains into chunk (0,0)'s score slots like every
    other batch's does, so scores start ~4us earlier.
  - last chunk's two per-head output DMAs go out on different engines
    (sync + scalar) so their issue costs overlap.

Measured dead ends (do not retry blindly): fp8 DoubleRow projections
(numerically fine with a 3-term main+residual scheme, rel-err 5.3e-3,
but DR matmuls run at ~1 cycle per MOVING row on HW = no speedup + more
LDWEIGHTS); v-transpose via dma_start_transpose (~1.2us per issue on the
sync queue, va tiles arrive ~1us late at every chunk); mid-drain partial
po copies (bounding-box PSUM hazards serialize the final AVs); final
DMA on gpsimd SWDGE (+4us, worse first-byte latency); N_WARM 26 + both
final DMAs serialized on sync + ACT/DVE-split final copy (+3.8us, cause
not isolated); N_WARM 12 (data-wait gap slips the clock grant).

Per-core per-chunk dataflow (unchanged from v2):
  qT/kT/vT [128, t] = W_slice @ query.T  (PE, bf16, K=E in 8 chunks, fp32
  PSUM, bias added on the psum->SBUF copy by the ACT)
  va [t(s), head, 65] = PE-transpose of vT, with a ones column at index 64.
  per (b, t-chunk c of 512):
    scoresT [s=128, t] per s-tile p (PE, skipping above-diagonal and
    fully-padded tiles); exp with per-partition padding bias (ACT);
    0/1 band mask on the diagonal 128 cols (DVE);
    po [65, 2, t] += va.T @ probs (PE; row 64 = denominator);
    po -> SBUF bf16 copy (DVE) -> DMA out.
  Output normalize (num/den) + transpose happens on the host in numpy.
"""

import numpy as np

T, B, E, H = 1024, 4, 1024, 16
D = 64  # head dim
NCORES = 8
HPC = H // NCORES  # heads per core = 2
DS = HPC * D  # per-core E-slice = 128
NEG = -1.0e30
SCALE = D**-0.5
N_WARM = 16  # PE warmup transposes (clock ramp + DMA-fill cover)

_COMPILED = {}


def _build_program(S):
    import concourse.bacc as bacc
    import concourse.mybir as mybir
    import concourse.tile as tile
    from concourse.masks import make_identity

    f32 = mybir.dt.float32
    bf16 = mybir.dt.bfloat16
    AF = mybir.ActivationFunctionType
    ALU = mybir.AluOpType

    nc = bacc.Bacc("TRN2", target_bir_lowering=False, debug=False,
                   num_devices=NCORES)

    # qt pre-tiled on the host: [chunk j, p, c, t] so each chunk DMA is one
    # contiguous 8KB run per partition (128 descriptors instead of 1024)
    qt = nc.dram_tensor("qt", [2 * B, 128, 8, 512], bf16,
                        kind="ExternalInput").ap()
    wq = nc.dram_tensor("wq", [128, 8, DS], bf16, kind="ExternalInput").ap()
    wk = nc.dram_tensor("wk", [128, 8, DS], bf16, kind="ExternalInput").ap()
    wv = nc.dram_tensor("wv", [128, 8, DS], bf16, kind="ExternalInput").ap()
    bqkv = nc.dram_tensor("bqkv", [DS, 3], f32, kind="ExternalInput").ap()
    band = nc.dram_tensor("band", [128, 128], bf16, kind="ExternalInput").ap()
    # additive exp bias per (s-partition, batch, key-tile): 0 for valid
    # keys, -1e30 for padded ones (exp -> 0)
    pad = nc.dram_tensor("pad", [128, B * 8], f32, kind="ExternalInput").ap()
    # [b, chunk, d-or-den, head, t] with the denominator in row 64
    out = nc.dram_tensor("out", [B, 2, 65, HPC, 512], bf16,
                         kind="ExternalOutput").ap()

    with tile.TileContext(nc) as tc:
        with (
            tc.tile_pool(name="consts", bufs=1) as consts,
            tc.tile_pool(name="qup", bufs=8) as qup,
            tc.tile_pool(name="qkp", bufs=2) as qkp,
            tc.tile_pool(name="vtp", bufs=2) as vtp,
            tc.tile_pool(name="vap", bufs=2) as vap,
            tc.tile_pool(name="prp", bufs=18) as prp,
            tc.tile_pool(name="posp", bufs=3) as posp,
            # psA double-buffered so the next projection unit's matmuls
            # don't wait the previous unit's ACT bias-copy; the transpose
            # tiles ride inside psS's "sc" tag footprint (tags are sized by
            # the max tile, so a [128,128]bf16 costs no extra bank there)
            tc.tile_pool(name="psA", bufs=2, space="PSUM") as psA,
            tc.tile_pool(name="psS", bufs=2, space="PSUM") as psS,
            tc.tile_pool(name="psO", bufs=1, space="PSUM") as psO,
        ):
            # ---- warmup stationary: memset only (no make_identity dep) so
            # the PE can start ramping the HAM clock immediately; gpsimd
            # finishes its preamble first so it does the memset ----
            warm = consts.tile([128, 128], bf16, name="warm")
            nc.gpsimd.memset(warm[:], 1.0)

            # ---- all input DMAs issued up-front on three queues ----
            w_sb = {}
            b_sb = {}
            for nm, wdr in (("q", wq), ("k", wk), ("v", wv)):
                w_sb[nm] = consts.tile([128, 8, DS], bf16, name=f"w{nm}s")
            bqkv_sb = consts.tile([DS, 3], f32, name="bqkvs")
            for i, nm in enumerate(("q", "k", "v")):
                b_sb[nm] = bqkv_sb[:, i:i + 1]
            band_sb = consts.tile([128, 128], bf16, name="bands")
            pad_sb = consts.tile([128, B * 8], f32, name="pads")

            qu_t = [qup.tile([128, 8, 512], bf16, tag="qu", name=f"qu{j}")
                    for j in range(2 * B)]
            # single sync ring, strict consumption order (the scalar HWDGE
            # queue starts ~4us later than sync, so putting anything
            # first-needed there stalls the first projection)
            nc.sync.dma_start(w_sb["q"][:], wq)
            nc.sync.dma_start(qu_t[0][:, 0:4, :], qt[0][:, 0:4, :])
            nc.sync.dma_start(bqkv_sb[:], bqkv)
            nc.sync.dma_start(qu_t[0][:, 4:8, :], qt[0][:, 4:8, :])
            nc.sync.dma_start(w_sb["k"][:], wk)
            nc.sync.dma_start(w_sb["v"][:], wv)
            for j in range(1, 2 * B):
                nc.sync.dma_start(qu_t[j][:], qt[j])
            # gpsimd (SWDGE): small mask constants
            nc.gpsimd.dma_start(band_sb[:], band)
            nc.gpsimd.dma_start(pad_sb[:], pad)

            # PE warmup: dummy transposes while the input DMAs are in
            # flight. Trips the HAM activity monitor so the real projection
            # starts at 2.4 GHz.
            for wi in range(N_WARM):
                tpw = psS.tile([128, 128], bf16, tag="sc", name=f"warm{wi}")
                nc.tensor.transpose(tpw[:], warm[:], warm[:])

            ident = consts.tile([128, 128], bf16, name="ident")
            make_identity(nc, ident[:])

            # ---- per-batch persistent tiles ----
            qk_t = {}   # (nm, b) -> [128, 1024] bf16
            vt_t = {}   # b -> [128, 1024] bf16 (vT, bias applied)
            va_t = {}   # b -> [128, 8, HPC, 65] bf16

            # ---- projection work units for batch b ----
            # Unit order: [set1, q-j0, k-j0, v-j0, vtr 0..3, q-j1, k-j1,
            # v-j1, vtr 4..] — all j0 units precede j1 units so a suffix of
            # the list can be deferred into a batch's own attention chunks
            # (used for batch 0 and the last batch).
            def proj_unit(nm, b, j, N, dst, nsub=1):
                def emit():
                    ps = psA.tile([128, 512], f32, tag="proj",
                                  name=f"ps{nm}{b}{j}")
                    qu = qu_t[2 * b + j]
                    for e in range(8):
                        nc.tensor.matmul(
                            ps[:, 0:N],
                            w_sb[nm][:, e, :],
                            qu[:, e, 0:N],
                            start=(e == 0),
                            stop=(e == 7),
                        )
                    nc.scalar.activation(
                        dst[:, 512 * j:512 * j + N], ps[:, 0:N],
                        AF.Identity, bias=b_sb[nm], scale=1.0,
                    )

                return emit

            def vtr_unit(b, i):
                def emit():
                    tp = psS.tile([128, 128], bf16, tag="sc",
                                  name=f"tp{b}_{i}")
                    nc.tensor.transpose(
                        tp[:], vt_t[b][:, 128 * i:128 * (i + 1)], ident[:],
                    )
                    nc.vector.tensor_copy(
                        va_t[b][:, i, :, 0:64],
                        tp[:].rearrange("p (two sub) -> p two sub", two=2),
                    )

                return emit

            def proj_units(b):
                dsts = {}
                for nm in ("q", "k"):
                    dsts[nm] = qkp.tile([128, T], bf16, tag=nm,
                                        name=f"{nm}{b}")
                    qk_t[(nm, b)] = dsts[nm]
                dsts["v"] = vtp.tile([128, T], bf16, tag="vt", name=f"vt{b}")
                vt_t[b] = dsts["v"]
                va = vap.tile([128, 8, HPC, 65], bf16, tag="va",
                              name=f"va{b}")
                va_t[b] = va
                units = [lambda: nc.vector.memset(va[:, :, :, 64:65], 1.0)]
                for j in range(2):
                    for nm in ("q", "k", "v"):
                        ncols = T if nm == "q" else min(T, 128 * S[b])
                        N = min(512, ncols - 512 * j)
                        if N > 0:
                            units.append(proj_unit(nm, b, j, N, dsts[nm]))
                    for i in range(4 * j, min(4 * (j + 1), S[b])):
                        units.append(vtr_unit(b, i))
                return units

            # ---- attention emission ----
            def emit_scores(b, c, p):
                w0 = max(0, 128 * (p - 4 * c))
                ss = psS.tile([128, HPC, 512], f32, tag="sc",
                              name=f"sc{b}_{c}_{p}")
                kt = qk_t[("k", b)]
                qt_b = qk_t[("q", b)]
                for hl in range(HPC):
                    nc.tensor.matmul(
                        ss[:, hl, w0:512],
                        kt[64 * hl:64 * hl + 64, 128 * p:128 * p + 128],
                        qt_b[64 * hl:64 * hl + 64,
                             512 * c + w0:512 * (c + 1)],
                        start=True,
                        stop=True,
                    )
                pr = prp.tile([128, HPC, 512], bf16, tag="pr",
                              name=f"pr{b}_{c}_{p}")
                nc.scalar.activation(
                    pr[:, :, w0:512],
                    ss[:, :, w0:512],
                    AF.Exp,
                    bias=pad_sb[:, b * 8 + p:b * 8 + p + 1],
                    scale=1.0,
                )
                dlt = p - 4 * c
                if dlt >= 0:
                    nc.vector.tensor_tensor(
                        pr[:, :, w0:w0 + 128],
                        pr[:, :, w0:w0 + 128],
                        band_sb[:, None, :].to_broadcast((128, HPC, 128)),
                        ALU.mult,
                    )
                return pr

            def emit_av(b, c, p, ntile, pr, po):
                w0 = max(0, 128 * (p - 4 * c))
                for hl in range(HPC):
                    nc.tensor.matmul(
                        po[:, hl, w0:512],
                        va_t[b][:, p, hl, :],
                        pr[:, hl, w0:512],
                        start=(p == 0),
                        stop=(p == ntile - 1),
                    )

            def emit_epilogue(b, c, po, last=False):
                pos = posp.tile([65, HPC, 512], bf16, tag="pos",
                                name=f"pos{b}_{c}")
                if last:
                    nc.scalar.copy(pos[:], po[:])
                else:
                    nc.vector.tensor_copy(pos[:], po[:])
                # last chunk: HWDGE (sync) has ~1.4us lower first-byte
                # latency than the gpsimd SWDGE path; mid-kernel chunks stay
                # on the idle gpsimd queue
                eng = nc.sync if last else nc.gpsimd
                eng.dma_start(out[b, c], pos[:])

            # ---------------- schedule ----------------
            # batch 0: only set1/q-j0/k-j0 run before attention (chunk
            # (0,0)'s scores need just those); the rest defers into the
            # chunk slots like every other batch's projection does.
            u0 = proj_units(0)
            for fn in u0[:3]:
                fn()
            punits = u0[3:]
            # units of u0[3:] that must complete before (0,1)'s scores:
            # v-j0, vtr0..min(4,S0)-1, q-j1, k-j1
            pops_before_01 = 3 + min(4, S[0])
            npops = [0]

            def pop_unit():
                punits.pop(0)()
                npops[0] += 1

            chunks = [(b, c) for b in range(B) for c in range(2)]
            prev = None           # (b, c, po)
            prev_pending = []     # [(b, c, p, ntile, pr, po)]
            punits_hold = []      # last batch's j1 units, deferred into its
            punits_hold_c1 = []
            for (b, c) in chunks:  # own attention chunks (pipeline drain)
                if c == 0:
                    if b > 0:
                        while punits:  # leftovers must land before b's
                            pop_unit()  # scores
                    if b + 1 < B:
                        nxt = proj_units(b + 1)
                        if b + 1 == B - 1:
                            # defer everything not needed by the last
                            # batch's own scores: v-j0+vtr0..3+q-j1+k-j1
                            # drain during its c=0 chunk, v-j1+vtr4.. during
                            # c=1 (AV consumption lags a chunk behind)
                            idx1 = 4 + min(4, S[b + 1])  # end of j0 block
                            punits_hold = nxt[3:idx1 + 2]
                            punits_hold_c1 = nxt[idx1 + 2:]
                            nxt = nxt[:3]
                        punits += nxt
                    else:
                        punits += punits_hold
                elif b == 0:
                    # (0,1): batch-0's q-j1/k-j1 must land before scores
                    while npops[0] < pops_before_01:
                        pop_unit()
                elif b + 1 >= B:
                    # last batch, c=1: its scores need q-j1/k-j1 NOW
                    while punits:
                        pop_unit()
                    punits += punits_hold_c1
                ntile = min(4 * (c + 1), S[b])
                last_chunk = (b, c) == chunks[-1]
                po = psO.tile([65, HPC, 512], f32, tag="po",
                              name=f"po{b}_{c}")
                prs = []
                for p in range(ntile):
                    pr = emit_scores(b, c, p)
                    if punits:
                        pop_unit()
                    if prev_pending:
                        emit_av(*prev_pending.pop(0))
                    if last_chunk:
                        # drain the previous chunk at 2 AVs/slot and lag our
                        # own AVs only 3 tiles behind: shortens the pipeline
                        # drain after the final scores tile
                        if prev_pending:
                            emit_av(*prev_pending.pop(0))
                        if not prev_pending and prev is not None:
                            emit_epilogue(*prev)
                            prev = None
                    elif p == 2 and prev is not None:
                        while prev_pending:
                            emit_av(*prev_pending.pop(0))
                        emit_epilogue(*prev)
                        prev = None
                    prs.append((b, c, p, ntile, pr, po))
                    if last_chunk and p >= 3:
                        emit_av(*prs[p - 3])
                if last_chunk:
                    while prev_pending:  # safety for tiny chunk counts
                        emit_av(*prev_pending.pop(0))
                    if prev is not None:
                        emit_epilogue(*prev)
                    rem = prs[max(0, ntile - 3):]
                    for args in rem[:-1]:
                        emit_av(*args)
                    # final tile: per-head AV -> copy -> DMA; the two DMA
                    # issues go on different engines so they overlap
                    (_, _, p_l, nt_l, pr_l, po_l) = rem[-1]
                    w0l = max(0, 128 * (p_l - 4 * c))
                    pos = posp.tile([65, HPC, 512], bf16, tag="pos",
                                    name=f"pos{b}_{c}")
                    for hl in range(HPC):
                        nc.tensor.matmul(
                            po_l[:, hl, w0l:512],
                            va_t[b][:, p_l, hl, :],
                            pr_l[:, hl, w0l:512],
                            start=(p_l == 0),
                            stop=True,
                        )
                        nc.scalar.copy(pos[:, hl, :], po_l[:, hl, :])
                        eng = nc.sync if hl == 0 else nc.scalar
                        eng.dma_start(out[b, c, :, hl, :], pos[:, hl, :])
                    prev = None
                    prev_pending = []
                else:
                    while prev_pending:
                        emit_av(*prev_pending.pop(0))
                        if punits:
                            pop_unit()
                    if prev is not None:
                        emit_epilogue(*prev)
                    prev = (b, c, po)
                    prev_pending = prs

    nc.compile()
    return nc


def _get_program(S):
    S = tuple(S)
    if S not in _COMPILED:
        _COMPILED[S] = _build_program(S)
    return _COMPILED[S]


def _prepare_in_maps(query, key_padding_mask, attn_mask, Wq, bq, Wk, bk, Wv,
                     bv):
    import ml_dtypes

    bf16 = ml_dtypes.bfloat16

    query = np.asarray(query, dtype=np.float32)
    attn_mask = np.asarray(attn_mask, dtype=np.float32)
    kpm = np.asarray(key_padding_mask)
    Wq, Wk, Wv = (np.asarray(w, dtype=np.float32) for w in (Wq, Wk, Wv))
    bq, bk, bv = (np.asarray(x, dtype=np.float32) for x in (bq, bk, bv))

    Wq = Wq * SCALE
    bq = bq * SCALE
    # [T, B, E] -> [E, B, T] -> [(8c, 128p), B, (2j2, 512t)] -> chunk-major
    # [B*2, 128, 8, 512] so each per-core chunk DMA is fully contiguous
    qt = query.transpose(2, 1, 0).reshape(8, 128, B, 2, 512)
    qt = np.ascontiguousarray(qt.transpose(2, 3, 1, 0, 4))
    qt = qt.reshape(2 * B, 128, 8, 512).astype(bf16)

    # number of 128-wide key tiles per batch that contain >= 1 valid key
    valid = ~kpm  # [B, T], True = usable key
    vt = valid.reshape(B, 8, 128).any(axis=2)  # [B, 8]
    S = []
    for b in range(B):
        nz = np.nonzero(vt[b])[0]
        S.append(int(nz[-1]) + 1 if len(nz) else 1)
    S = tuple(S)

    # 0/1 multiplicative causal mask for the diagonal 128x128 band:
    # band[ps, ft] = 1 iff attn_mask[ft, ps] is finite (ft, ps < 128)
    band = np.isfinite(attn_mask[:128, :128].T).astype(np.float32)
    band = np.ascontiguousarray(band).astype(bf16)

    padf = np.where(kpm, NEG, 0.0).astype(np.float32)  # [B, T]
    padh = np.ascontiguousarray(
        padf.reshape(B, 8, 128).transpose(2, 0, 1).reshape(128, B * 8)
    )

    def wtile(w):  # [DS, E] -> [E, DS] -> [128p, 8c, DS] pre-tiled
        wt = w.T.reshape(8, 128, DS).transpose(1, 0, 2)
        return np.ascontiguousarray(wt).astype(bf16)

    in_maps = []
    for i in range(NCORES):
        rows = slice(i * DS, (i + 1) * DS)
        in_maps.append(
            {
                "qt": qt,
                "wq": wtile(Wq[rows]),
                "wk": wtile(Wk[rows]),
                "wv": wtile(Wv[rows]),
                "bqkv": np.ascontiguousarray(
                    np.stack([bq[rows], bk[rows], bv[rows]], axis=1)),
                "band": band,
                "pad": padh,
            }
        )
    return in_maps, S


def kernel(query, key, key_padding_mask, attn_mask, Wq, bq, Wk, bk, Wv, bv,
           num_heads):
    from concourse.bass_utils import run_bass_kernel_spmd

    assert int(num_heads) == H
    in_maps, S = _prepare_in_maps(query, key_padding_mask, attn_mask, Wq, bq,
                                  Wk, bk, Wv, bv)
    nc = _get_program(S)
    res = run_bass_kernel_spmd(nc, in_maps, core_ids=list(range(NCORES)))
    # res: per-core out [B, 2, 65, HPC, 512] bf16
    full = np.empty((T, B, E), dtype=np.float32)
    for i in range(NCORES):
        arr = np.asarray(res.results[i]["out"], dtype=np.float32)
        num = arr[:, :, 0:64]        # [B, C, D, HL, TL]
        den = arr[:, :, 64:65]       # [B, C, 1, HL, TL]
        r = num / den                # [B, C, D, HL, TL]
        # -> [C, TL, B, HL, D] -> [T, B, 128]
        r = r.transpose(1, 4, 0, 3, 2).reshape(T, B, DS)
        full[:, :, i * DS:(i + 1) * DS] = r
    return full
